# revision 1
# baseline (speedup 1.0000x reference)
"""TRN2 Bass kernel for nn_CharModel (segment-mean over char ranges + pos embedding).

Strategy (pure data-parallel over batch, 8 cores x 4 batches):
  - Host computes per-word [start, end) ranges exactly as the reference does and
    sorts each batch's words by length descending, so every gather step's
    participant set ("len >= threshold") is a dense slot prefix.
  - Device: per batch a few SWDGE dma_gather steps land char rows in slot order
    [p = i%128, chunk = i//128]:
      * odd step first: row start+len-1 for odd-len words gathered straight
        into the accumulator (even-len / padding words point at zeros rows
        appended to the per-core feats copy, so every slot is initialized)
      * pair step t: rows (start+2t, start+2t+1) for words with len >= 2t+2 --
        one 6KB descriptor per word (elem_size=2*D over an elem_step=D
        overlapping view); DVE prefix-adds fold the stages into the accumulator.
    Total gathered bytes ~= one pass over feats (memory roofline).
  - Pos embedding via a host-built one-hot matmul on PE (PSUM), fused with the
    1/len scaling in one scalar_tensor_tensor per 128-word chunk:
       out = (acc * recip) + psum_pos
  - Host unpermutes word slots and stacks cores.
"""

import numpy as np

B, S, W, D, PV = 32, 2048, 512, 768, 64
N_CORES = 8
BPC = B // N_CORES          # batches per core
P = 128
C = W // P                  # 4 word-chunks per batch
ZROW = BPC * S              # first zeros row in feats_cat
NZROWS = 40                 # spread pad reads across many zero rows
KMAX_DEVICE = 48            # device path supports word len up to this

LAST_RESULTS = None         # BassKernelResults of the most recent run (for test.py)


def _run_spmd(nc, in_maps, core_ids):
    """Indirection point so tests can swap in a simulator."""
    from concourse.bass_utils import run_bass_kernel_spmd
    return run_bass_kernel_spmd(nc, in_maps, core_ids)


def _word_ranges(word_lens, pos, seq_len):
    """Replicate the reference's starts/ends/valid computation in numpy."""
    wl = np.asarray(word_lens, np.int64)
    po = np.asarray(pos, np.int64)
    sl = np.asarray(seq_len, np.int64)
    b, w = wl.shape
    j = np.arange(w)
    next_start = np.concatenate([wl[:, 1:], np.zeros((b, 1), np.int64)], axis=1)
    is_last = (j[None, :] == w - 1) | (next_start == 0)
    starts = wl
    ends = np.where(is_last, sl[:, None], next_start)
    valid = (wl != 0) | (j[None, :] == 0)
    lens = np.where(valid, np.maximum(ends - starts, 0), 0)
    denom = np.maximum(ends - starts, 1).astype(np.float64)
    recip = np.where(valid & (lens > 0), 1.0 / denom, 0.0).astype(np.float32)
    return starts, lens, recip, po


def _numpy_fallback(feats, pos_table, word_lens, pos, seq_len):
    feats = np.asarray(feats, np.float32)
    pos_table = np.asarray(pos_table, np.float32)
    starts, lens, recip, po = _word_ranges(word_lens, pos, seq_len)
    out = np.zeros((feats.shape[0], po.shape[1], feats.shape[2]), np.float32)
    for b in range(out.shape[0]):
        for w in range(out.shape[1]):
            L = int(lens[b, w])
            if L > 0:
                s = int(starts[b, w])
                out[b, w] = feats[b, s:s + L].sum(axis=0) * recip[b, w]
        out[b] += pos_table[po[b]]
    return out


def _wrap16(flat):
    """int16 flat index list [W] -> the q7 kernel's [16, W/16] wrapped layout,
    replicated across the 8 q7 core stripes (128 partitions)."""
    wrapped = flat.astype(np.int16).reshape(-1, 16).T
    return np.tile(wrapped, (8, 1))


def _concourse_importable():
    try:
        import concourse.bass  # noqa: F401
        return True
    except ImportError:
        import sys
        for p in ("/opt/trn_rl_repo", "/root/.axon_site/_ro/trn_rl_repo"):
            if p not in sys.path:
                sys.path.append(p)
        try:
            import concourse.bass  # noqa: F401
            return True
        except ImportError:
            return False


def kernel(feats, pos_table, word_lens, pos, seq_len):
    global LAST_RESULTS
    feats = np.ascontiguousarray(np.asarray(feats, np.float32))
    pos_table_np = np.ascontiguousarray(np.asarray(pos_table, np.float32))
    starts, lens, recip, po = _word_ranges(word_lens, pos, seq_len)

    kmax = int(lens.max())
    shapes_ok = (
        feats.shape == (B, S, D)
        and pos_table_np.shape == (PV, D)
        and po.shape == (B, W)
        and starts.shape == (B, W)
        and np.asarray(seq_len).shape == (B,)
        and int(po.max()) < PV and int(po.min()) >= 0
    )
    if kmax > KMAX_DEVICE or not shapes_ok or not _concourse_importable():
        return _numpy_fallback(feats, pos_table, word_lens, pos, seq_len)
    kmax = max(kmax, 1)
    n_pair = kmax // 2                         # pair step t covers rows 2t,2t+1

    # ---- host-side per-core tensors -------------------------------------
    perms = np.zeros((B, W), np.int64)                 # slot i -> word perms[b, i]
    pair_n = np.zeros((B, max(n_pair, 1)), np.int64)   # words with len >= 2t+2
    any_odd = np.zeros(B, bool)
    for b in range(B):
        perm = np.argsort(-lens[b], kind="stable")
        perms[b] = perm
        sl = lens[b][perm]
        for t in range(n_pair):
            pair_n[b, t] = int((sl >= 2 * t + 2).sum())
        any_odd[b] = bool((lens[b] % 2 == 1).any())
    pair_n_u = pair_n.reshape(N_CORES, BPC, -1).max(axis=0)   # [BPC, n_pair]
    odd_u = any_odd.reshape(N_CORES, BPC).any(axis=0)          # [BPC]

    idx_cols = W // 16                         # 32 int16 columns per step
    n_steps = n_pair + 1                       # odd step slot + pair steps
    in_maps = []
    host_meta = []
    for core in range(N_CORES):
        bs = slice(core * BPC, (core + 1) * BPC)
        feats_cat = np.zeros((BPC * S + NZROWS, D), np.float32)
        feats_cat[:BPC * S] = feats[bs].reshape(-1, D)

        idx_all = np.full((128, BPC * n_steps * idx_cols), -1, np.int16)
        recip_all = np.zeros((P, BPC * C), np.float32)
        onehot_all = np.zeros((PV, BPC * W), np.float32)
        for bl in range(BPC):
            bg = core * BPC + bl
            perm = perms[bg]
            st = starts[bg][perm]
            ln = lens[bg][perm]
            odd = (ln % 2 == 1)
            zcycle = ZROW + (np.arange(W) % NZROWS)
            flat = np.where(odd, bl * S + st + ln - 1, zcycle)
            col0 = (bl * n_steps) * idx_cols
            idx_all[:, col0:col0 + idx_cols] = _wrap16(flat)
            for t in range(n_pair):
                nn = int(pair_n_u[bl, t])
                if nn == 0:
                    continue
                nv = int(pair_n[bg, t])
                flat = np.full(W, -1, np.int64)
                flat[:nv] = bl * S + st[:nv] + 2 * t
                flat[nv:nn] = ZROW + (np.arange(nn - nv) % (NZROWS - 1))
                col0 = (bl * n_steps + 1 + t) * idx_cols
                idx_all[:, col0:col0 + idx_cols] = _wrap16(flat)
            slot_r = recip[bg][perm]              # [512] in slot order
            recip_all[:, bl * C:(bl + 1) * C] = slot_r.reshape(C, P).T
            slot_pos = po[bg][perm]
            onehot_all[slot_pos, bl * W + np.arange(W)] = 1.0

        # batch-0 odd-step indices in [128, C] int32 column layout for the
        # library-free indirect gather that warms up under the q7 library load
        bg0 = core * BPC
        perm0 = perms[bg0]
        ln0 = lens[bg0][perm0]
        st0 = starts[bg0][perm0]
        odd0 = (ln0 % 2 == 1)
        zc0 = ZROW + (np.arange(W) % NZROWS)
        flat0 = np.where(odd0, st0 + ln0 - 1, zc0)
        odd0_idx = flat0.reshape(C, P).T.astype(np.int32)

        in_maps.append({
            "feats_cat": feats_cat,
            "pos_tab": pos_table_np,
            "idx_all": idx_all,
            "recip_all": recip_all,
            "onehot_all": onehot_all,
            "odd0_idx": odd0_idx,
        })
        host_meta.append(perms[bs])

    # ---- device program --------------------------------------------------
    from concourse import bass, bacc, mybir
    import concourse.tile as tile

    nc = bacc.Bacc("TRN2", target_bir_lowering=False, debug=False)
    t_feats = nc.dram_tensor("feats_cat", [BPC * S + NZROWS, D], mybir.dt.float32,
                             kind="ExternalInput")
    t_pos = nc.dram_tensor("pos_tab", [PV, D], mybir.dt.float32,
                           kind="ExternalInput")
    t_idx = nc.dram_tensor("idx_all", [128, BPC * n_steps * idx_cols],
                           mybir.dt.int16, kind="ExternalInput")
    t_recip = nc.dram_tensor("recip_all", [P, BPC * C], mybir.dt.float32,
                             kind="ExternalInput")
    t_oh = nc.dram_tensor("onehot_all", [PV, BPC * W], mybir.dt.float32,
                          kind="ExternalInput")
    t_odd0 = nc.dram_tensor("odd0_idx", [P, C], mybir.dt.int32,
                            kind="ExternalInput")
    t_out = nc.dram_tensor("out", [BPC, P, C * D], mybir.dt.float32,
                           kind="ExternalOutput")

    # overlapping pair view: index i -> 2*D consecutive elements (rows i, i+1)
    feats_ap = t_feats[:]
    pair_src = bass.AP(feats_ap.tensor, 0,
                       [[D, BPC * S + NZROWS - 1], [1, 2 * D]])

    with tile.TileContext(nc) as tc:
        with (
            tc.tile_pool(name="const", bufs=1) as cpool,
            tc.tile_pool(name="work", bufs=3) as wpool,
            tc.tile_pool(name="stage", bufs=5) as spool,
            tc.tile_pool(name="psum", bufs=4, space="PSUM") as ppool,
        ):
            pos_sb = cpool.tile([PV, D], mybir.dt.float32)
            oh_sb = cpool.tile([PV, BPC * W], mybir.dt.float32)
            recip_sb = cpool.tile([P, BPC * C], mybir.dt.float32)
            idx_sb = cpool.tile([128, BPC * n_steps * idx_cols], mybir.dt.int16)
            odd0_sb = cpool.tile([P, C], mybir.dt.int32)
            nc.sync.dma_start(out=odd0_sb[:], in_=t_odd0[:])
            nc.sync.dma_start(out=idx_sb[:], in_=t_idx[:])
            nc.sync.dma_start(out=pos_sb[:], in_=t_pos[:])
            nc.sync.dma_start(out=oh_sb[:], in_=t_oh[:])
            nc.sync.dma_start(out=recip_sb[:], in_=t_recip[:])

            accs = {}

            def issue_odd(bl):
                acc = wpool.tile([P, C, D], mybir.dt.float32, tag="acc")
                accs[bl] = acc
                if bl == 0:
                    # library-free indirect gathers: run while the q7 dma_gather
                    # library is still being fetched
                    for c in range(C):
                        nc.gpsimd.indirect_dma_start(
                            out=acc[:, c, :],
                            out_offset=None,
                            in_=t_feats[:],
                            in_offset=bass.IndirectOffsetOnAxis(
                                ap=odd0_sb[:, c:c + 1], axis=0
                            ),
                        )
                    return
                nc.gpsimd.dma_gather(
                    acc[:],
                    t_feats[:],
                    idx_sb[:, bl * n_steps * idx_cols:
                           bl * n_steps * idx_cols + idx_cols],
                    W, W, D, single_packet=False,
                )

            def issue_pair(bl, t):
                nn = int(pair_n_u[bl, t])
                if nn == 0:
                    return
                acc = accs[bl]
                stg = spool.tile([P, C, 2 * D], mybir.dt.float32, tag="stg2")
                colk = (bl * n_steps + 1 + t) * idx_cols
                nc.gpsimd.dma_gather(
                    stg[:],
                    pair_src,
                    idx_sb[:, colk:colk + idx_cols],
                    W, nn, 2 * D, elem_step=D, single_packet=False,
                )
                fc, rem = nn // P, nn % P
                if fc:
                    nc.vector.tensor_add(
                        out=acc[:, 0:fc, :], in0=acc[:, 0:fc, :],
                        in1=stg[:, 0:fc, 0:D],
                    )
                    nc.vector.tensor_add(
                        out=acc[:, 0:fc, :], in0=acc[:, 0:fc, :],
                        in1=stg[:, 0:fc, D:2 * D],
                    )
                if rem:
                    nc.vector.tensor_add(
                        out=acc[0:rem, fc, :], in0=acc[0:rem, fc, :],
                        in1=stg[0:rem, fc, 0:D],
                    )
                    nc.vector.tensor_add(
                        out=acc[0:rem, fc, :], in0=acc[0:rem, fc, :],
                        in1=stg[0:rem, fc, D:2 * D],
                    )

            def issue_epilogue(bl):
                acc = accs[bl]
                for c in range(C):
                    psum = ppool.tile([P, D], mybir.dt.float32, space="PSUM",
                                      tag="psum")
                    lhs = oh_sb[:, bl * W + c * P: bl * W + (c + 1) * P]
                    nc.tensor.matmul(out=psum[:, 0:512], lhsT=lhs,
                                     rhs=pos_sb[:, 0:512], start=True, stop=True)
                    nc.tensor.matmul(out=psum[:, 512:D], lhsT=lhs,
                                     rhs=pos_sb[:, 512:D], start=True, stop=True)
                    nc.vector.scalar_tensor_tensor(
                        out=acc[:, c, :],
                        in0=acc[:, c, :],
                        scalar=recip_sb[:, bl * C + c: bl * C + c + 1],
                        in1=psum[:],
                        op0=mybir.AluOpType.mult,
                        op1=mybir.AluOpType.add,
                    )
                    # store each chunk as soon as its epilogue op retires, so
                    # only a 384KB DMA (not 1.5MB) trails the last compute
                    nc.sync.dma_start(
                        out=t_out[bl, :, c * D:(c + 1) * D],
                        in_=acc[:, c, :],
                    )

            # Batches 0..BPC-3 run batch-major; the last two batches interleave
            # their gathers so the final batch's add-chain starts while the
            # remaining gathers drain, shrinking the end-of-kernel tail.
            for bl in range(BPC - 2):
                issue_odd(bl)
                for t in range(n_pair):
                    issue_pair(bl, t)
                issue_epilogue(bl)
            if BPC >= 2:
                a, b = BPC - 2, BPC - 1
                issue_odd(a)
                issue_pair(a, 0)
                issue_odd(b)
                issue_pair(b, 0)
                for t in range(1, n_pair):
                    issue_pair(a, t)
                issue_epilogue(a)
                for t in range(1, n_pair):
                    issue_pair(b, t)
                issue_epilogue(b)
    nc.finalize()

    res = _run_spmd(nc, in_maps, list(range(N_CORES)))
    LAST_RESULTS = res

    out = np.empty((B, W, D), np.float32)
    for core in range(N_CORES):
        arr = res.results[core]["out"]            # [BPC, 128, C*D]
        for bl in range(BPC):
            slots = arr[bl].reshape(P, C, D).transpose(1, 0, 2).reshape(W, D)
            perm = host_meta[core][bl]
            out[core * BPC + bl][perm] = slots
    return out



# revision 12
# speedup vs baseline: 1.1626x; 1.1626x over previous
"""TRN2 Bass kernel for nn_CharModel (segment-mean over char ranges + pos embedding).

Strategy (pure data-parallel over batch, 8 cores x 4 batches):
  - Words are contiguous char ranges [start, start+L). Host sorts each batch's
    words by length L desc; same-length words across the core's 4 batches are
    packed (bl-major) into 128-partition columns. Per length L ONE
    indirect_dma_start call gathers every word's L rows as a single contiguous
    descriptor (L*768 bf16 elements via an overlapping row view) into
    [128, ncols_L, L*768] -- ~2.7k descriptors per core on the library-free
    INDIRECT1D q7 path, full-width so they spread evenly over all 16 SDMA
    engines.
  - feats is cast to bf16 on the host, halving HBM read traffic; sums
    accumulate in fp32 on DVE (L-1 adds per column), so only the per-element
    bf16 input rounding (~2^-9 relative) is lost.
  - SPMD runs one program on 8 cores: only the column COUNT per L is unified
    (max over cores); which word sits in which slot is per-core input data.
    Pad slots gather real rows (cheap, valid) and are neutralized by recip=0
    and a zero one-hot column; their output rows are discarded on the host.
  - Pos embedding via a host-built one-hot bf16 matmul on PE (PSUM), fused
    with the 1/len scaling in one scalar_tensor_tensor per column:
       out = (acc * recip) + psum_pos
  - Each column's finished [cu, 768] fp32 block DMAs straight to its row
    range of a flat output tensor; host scatters rows back to (batch, word)
    order. Rows the device never computes (len-0/invalid words) are exactly
    the pos-embedding row, filled on the host from the fp32 table.
"""

import numpy as np

B, S, W, D, PV = 32, 2048, 512, 768, 64
N_CORES = 8
BPC = B // N_CORES          # batches per core
P = 128
KMAX_DEVICE = 16            # device path supports word len up to this

LAST_RESULTS = None         # BassKernelResults of the most recent run (for test.py)


def _run_spmd(nc, in_maps, core_ids):
    """Indirection point so tests can swap in a simulator."""
    from concourse.bass_utils import run_bass_kernel_spmd
    return run_bass_kernel_spmd(nc, in_maps, core_ids)


def _word_ranges(word_lens, pos, seq_len):
    """Replicate the reference's starts/ends/valid computation in numpy."""
    wl = np.asarray(word_lens, np.int64)
    po = np.asarray(pos, np.int64)
    sl = np.asarray(seq_len, np.int64)
    b, w = wl.shape
    j = np.arange(w)
    next_start = np.concatenate([wl[:, 1:], np.zeros((b, 1), np.int64)], axis=1)
    is_last = (j[None, :] == w - 1) | (next_start == 0)
    starts = wl
    ends = np.where(is_last, sl[:, None], next_start)
    valid = (wl != 0) | (j[None, :] == 0)
    lens = np.where(valid, np.maximum(ends - starts, 0), 0)
    denom = np.maximum(ends - starts, 1).astype(np.float64)
    recip = np.where(valid & (lens > 0), 1.0 / denom, 0.0).astype(np.float32)
    return starts, lens, recip, po


def _numpy_fallback(feats, pos_table, word_lens, pos, seq_len):
    feats = np.asarray(feats, np.float32)
    pos_table = np.asarray(pos_table, np.float32)
    starts, lens, recip, po = _word_ranges(word_lens, pos, seq_len)
    out = np.zeros((feats.shape[0], po.shape[1], feats.shape[2]), np.float32)
    for b in range(out.shape[0]):
        for w in range(out.shape[1]):
            L = int(lens[b, w])
            if L > 0:
                s = int(starts[b, w])
                out[b, w] = feats[b, s:s + L].sum(axis=0) * recip[b, w]
        out[b] += pos_table[po[b]]
    return out


def _concourse_importable():
    try:
        import concourse.bass  # noqa: F401
        return True
    except ImportError:
        import sys
        for p in ("/opt/trn_rl_repo", "/root/.axon_site/_ro/trn_rl_repo"):
            if p not in sys.path:
                sys.path.append(p)
        try:
            import concourse.bass  # noqa: F401
            return True
        except ImportError:
            return False


def _prepare(feats, pos_table_np, starts, lens, recip, po, kmax):
    """Host-side layout.

    Returns (geom, in_maps, meta, tot_rows):
      geom: list of (L, colbase, ncols, cu_list) in descending-L order —
            the shared program shape.
      meta[core]: extraction records (bg, word_idx_array, out_row_start).
    """
    import ml_dtypes
    bf16 = ml_dtypes.bfloat16

    perms = np.zeros((B, W), np.int64)
    for b in range(B):
        perms[b] = np.argsort(-lens[b], kind="stable")
    # per (core, L): words bl-major in sorted order
    SL = np.zeros((N_CORES, kmax + 1), np.int64)
    for c in range(N_CORES):
        for L in range(1, kmax + 1):
            SL[c, L] = int((lens[c * BPC:(c + 1) * BPC] == L).sum())
    MS = SL.max(axis=0)                      # unified capacity per L

    geom = []
    colbase = 0
    rowbase = 0
    for L in range(kmax, 0, -1):
        if MS[L] == 0:
            continue
        ms = int(MS[L])
        ncols = -(-ms // P)
        cu_list = [min(P, ms - c * P) for c in range(ncols)]
        geom.append((L, colbase, ncols, cu_list, rowbase, ms))
        colbase += ncols
        rowbase += ms
    ncol_total = colbase
    tot_rows = rowbase

    in_maps = []
    meta = []
    for core in range(N_CORES):
        bs = slice(core * BPC, (core + 1) * BPC)
        feats_bf = feats[bs].reshape(-1, D).astype(bf16)
        # int16 wrapped gather indices: per L a [128, 8*ncols] block where
        # element (p, c) = flat[c*16 + p%16] (16-wrapped, replicated x8 cores)
        idx_all = np.full((P, 8 * ncol_total), -1, np.int16)
        recip_all = np.zeros((P, ncol_total), np.float32)
        oh_f = np.zeros((PV, ncol_total * P), np.float32)
        recs = []
        for (L, cb, ncols, cu_list, rb, ms) in geom:
            cap = P * ncols
            # slots [0, ms): valid rows (pads spread over low batch-0 rows);
            # slots [ms, cap): -1, never reached (num_idxs_reg = ms)
            flat = np.full(cap, -1, np.int64)
            flat[:ms] = (np.arange(ms) * 53) % (S - KMAX_DEVICE)
            slot = 0                         # slot index within this L block
            for bl in range(BPC):
                bg = core * BPC + bl
                perm = perms[bg]
                lsort = lens[bg][perm]
                gstart = int(np.searchsorted(-lsort, -L, side="left"))
                n_here = int((lens[bg] == L).sum())
                if n_here == 0:
                    continue
                wsel = perm[gstart:gstart + n_here]
                sl_idx = slot + np.arange(n_here)
                flat[sl_idx] = bl * S + starts[bg][wsel]
                pcol = sl_idx % P
                ccol = sl_idx // P
                recip_all[pcol, cb + ccol] = recip[bg][wsel]
                oh_f[po[bg][wsel], (cb + ccol) * P + pcol] = 1.0
                recs.append((bg, wsel, rb + slot))
                slot += n_here
            wrapped = flat.astype(np.int16).reshape(-1, 16).T   # [16, cap/16]
            idx_all[:, 8 * cb: 8 * (cb + ncols)] = np.tile(wrapped, (8, 1))
        in_maps.append({
            "feats_bf": feats_bf,
            "pos_tab": pos_table_np.astype(bf16),
            "idx_all": idx_all,
            "recip_all": recip_all,
            "oh_all": oh_f.astype(bf16),
        })
        meta.append(recs)
    return geom, ncol_total, in_maps, meta, tot_rows


def _build_nc(geom, ncol_total, tot_rows, kmax):
    from concourse import bass, bacc, mybir
    import concourse.tile as tile

    nrows = BPC * S
    nc = bacc.Bacc("TRN2", target_bir_lowering=False, debug=False)
    t_feats = nc.dram_tensor("feats_bf", [nrows, D], mybir.dt.bfloat16,
                             kind="ExternalInput")
    t_pos = nc.dram_tensor("pos_tab", [PV, D], mybir.dt.bfloat16,
                           kind="ExternalInput")
    t_idx = nc.dram_tensor("idx_all", [P, 8 * ncol_total], mybir.dt.int16,
                           kind="ExternalInput")
    t_recip = nc.dram_tensor("recip_all", [P, ncol_total], mybir.dt.float32,
                             kind="ExternalInput")
    t_oh = nc.dram_tensor("oh_all", [PV, ncol_total * P], mybir.dt.bfloat16,
                          kind="ExternalInput")
    t_out = nc.dram_tensor("out", [tot_rows, D], mybir.dt.float32,
                           kind="ExternalOutput")

    # overlapping row views: index i -> L*D consecutive elements (rows i..i+L-1)
    views = {}
    for (L, *_rest) in geom:
        views[L] = bass.AP(t_feats[:].tensor, 0,
                           [[D, nrows - L + 1], [1, L * D]])

    with tile.TileContext(nc) as tc:
        with (
            tc.tile_pool(name="const", bufs=1) as cpool,
            tc.tile_pool(name="gath", bufs=1) as gpool,
            tc.tile_pool(name="red", bufs=4) as rpool,
            tc.tile_pool(name="psum", bufs=4, space="PSUM") as ppool,
        ):
            idx_sb = cpool.tile([P, 8 * ncol_total], mybir.dt.int16)
            recip_sb = cpool.tile([P, ncol_total], mybir.dt.float32)
            pos_sb = cpool.tile([PV, D], mybir.dt.bfloat16)
            oh_sb = cpool.tile([PV, ncol_total * P], mybir.dt.bfloat16)
            nc.sync.dma_start(out=idx_sb[:], in_=t_idx[:])
            nc.sync.dma_start(out=recip_sb[:], in_=t_recip[:])
            nc.sync.dma_start(out=pos_sb[:], in_=t_pos[:])
            nc.sync.dma_start(out=oh_sb[:], in_=t_oh[:])

            gts = {}
            for (L, cb, ncols, cu_list, rb, ms) in geom:
                gt = gpool.tile([P, ncols, L * D], mybir.dt.bfloat16,
                                tag=f"g{L}")
                gts[L] = gt
                nc.gpsimd.dma_gather(
                    gt[:, :, :],
                    views[L],
                    idx_sb[:, 8 * cb: 8 * (cb + ncols)],
                    P * ncols,            # num_idxs (idx-buffer capacity)
                    ms,                   # num_idxs_reg (descriptors emitted)
                    L * D,
                    elem_step=D,
                    single_packet=False,
                )

            for (L, cb, ncols, cu_list, rb, ms) in geom:
                gt = gts[L]
                rowoff = rb
                for c in range(ncols):
                    cu = cu_list[c]
                    k = cb + c
                    rg = rpool.tile([P, D], mybir.dt.float32, tag="rg")
                    if L == 1:
                        nc.vector.tensor_copy(out=rg[0:cu, :],
                                              in_=gt[0:cu, c, 0:D])
                    else:
                        nc.vector.tensor_add(out=rg[0:cu, :],
                                             in0=gt[0:cu, c, 0:D],
                                             in1=gt[0:cu, c, D:2 * D])
                        for r in range(2, L):
                            nc.vector.tensor_add(
                                out=rg[0:cu, :], in0=rg[0:cu, :],
                                in1=gt[0:cu, c, r * D:(r + 1) * D])
                    psum = ppool.tile([P, D], mybir.dt.float32, space="PSUM",
                                      tag="ps")
                    lhs = oh_sb[:, k * P:k * P + cu]
                    nc.tensor.matmul(out=psum[0:cu, 0:512], lhsT=lhs,
                                     rhs=pos_sb[:, 0:512], start=True,
                                     stop=True)
                    nc.tensor.matmul(out=psum[0:cu, 512:D], lhsT=lhs,
                                     rhs=pos_sb[:, 512:D], start=True,
                                     stop=True)
                    nc.vector.scalar_tensor_tensor(
                        out=rg[0:cu, :],
                        in0=rg[0:cu, :],
                        scalar=recip_sb[0:cu, k:k + 1],
                        in1=psum[0:cu, :],
                        op0=mybir.AluOpType.mult,
                        op1=mybir.AluOpType.add,
                    )
                    nc.sync.dma_start(out=t_out[rowoff:rowoff + cu, :],
                                      in_=rg[0:cu, :])
                    rowoff += cu
    nc.finalize()
    return nc


def kernel(feats, pos_table, word_lens, pos, seq_len):
    global LAST_RESULTS
    feats = np.ascontiguousarray(np.asarray(feats, np.float32))
    pos_table_np = np.ascontiguousarray(np.asarray(pos_table, np.float32))
    starts, lens, recip, po = _word_ranges(word_lens, pos, seq_len)

    kmax = int(lens.max())
    shapes_ok = (
        feats.shape == (B, S, D)
        and pos_table_np.shape == (PV, D)
        and po.shape == (B, W)
        and starts.shape == (B, W)
        and np.asarray(seq_len).shape == (B,)
        and int(po.max()) < PV and int(po.min()) >= 0
    )
    if kmax > KMAX_DEVICE or kmax < 1 or not shapes_ok \
            or not _concourse_importable():
        return _numpy_fallback(feats, pos_table, word_lens, pos, seq_len)

    geom, ncol_total, in_maps, meta, tot_rows = _prepare(
        feats, pos_table_np, starts, lens, recip, po, kmax)
    nc = _build_nc(geom, ncol_total, tot_rows, kmax)

    res = _run_spmd(nc, in_maps, list(range(N_CORES)))
    LAST_RESULTS = res

    out = np.zeros((B, W, D), np.float32)
    for core in range(N_CORES):
        arr = res.results[core]["out"]            # [tot_rows, D]
        for bg, wsel, rowstart in meta[core]:
            out[bg][wsel] = arr[rowstart:rowstart + len(wsel)]
    # slots the device never computes: invalid words and len-0 words get
    # means == 0, so the exact answer is just the pos embedding row
    zmask = lens == 0
    if zmask.any():
        out[zmask] = pos_table_np[po[zmask]]
    return out


# revision 17
# speedup vs baseline: 1.5303x; 1.3162x over previous
"""TRN2 Bass kernel for nn_CharModel (segment-mean over char ranges + pos embedding).

Strategy (pure data-parallel over batch, 8 cores x 4 batches):
  - Words are contiguous char ranges [start, start+L). Host sorts each batch's
    words by length L desc; same-length words across the core's 4 batches are
    packed (bl-major) into 128-partition columns. Per length L ONE
    indirect_dma_start call gathers every word's L rows as a single contiguous
    descriptor (L*768 bf16 elements via an overlapping row view) into
    [128, ncols_L, L*768] -- ~2.7k descriptors per core on the library-free
    INDIRECT1D q7 path, full-width so they spread evenly over all 16 SDMA
    engines.
  - feats is cast to bf16 on the host, halving HBM read traffic; sums
    accumulate in fp32 on DVE (L-1 adds per column), so only the per-element
    bf16 input rounding (~2^-9 relative) is lost.
  - SPMD runs one program on 8 cores: only the column COUNT per L is unified
    (max over cores); which word sits in which slot is per-core input data.
    Pad slots gather real rows (cheap, valid) and are neutralized by recip=0
    and a zero one-hot column; their output rows are discarded on the host.
  - Pos embedding via a host-built one-hot bf16 matmul on PE (PSUM), fused
    with the 1/len scaling in one scalar_tensor_tensor per column:
       out = (acc * recip) + psum_pos
  - Each column's finished [cu, 768] fp32 block DMAs straight to its row
    range of a flat output tensor; host scatters rows back to (batch, word)
    order. Rows the device never computes (len-0/invalid words) are exactly
    the pos-embedding row, filled on the host from the fp32 table.
"""

import numpy as np

B, S, W, D, PV = 32, 2048, 512, 768, 64
N_CORES = 8
BPC = B // N_CORES          # batches per core
P = 128
KMAX_DEVICE = 16            # device path supports word len up to this

LAST_RESULTS = None         # BassKernelResults of the most recent run (for test.py)


def _run_spmd(nc, in_maps, core_ids):
    """Indirection point so tests can swap in a simulator."""
    from concourse.bass_utils import run_bass_kernel_spmd
    return run_bass_kernel_spmd(nc, in_maps, core_ids)


def _word_ranges(word_lens, pos, seq_len):
    """Replicate the reference's starts/ends/valid computation in numpy."""
    wl = np.asarray(word_lens, np.int64)
    po = np.asarray(pos, np.int64)
    sl = np.asarray(seq_len, np.int64)
    b, w = wl.shape
    j = np.arange(w)
    next_start = np.concatenate([wl[:, 1:], np.zeros((b, 1), np.int64)], axis=1)
    is_last = (j[None, :] == w - 1) | (next_start == 0)
    starts = wl
    ends = np.where(is_last, sl[:, None], next_start)
    valid = (wl != 0) | (j[None, :] == 0)
    lens = np.where(valid, np.maximum(ends - starts, 0), 0)
    denom = np.maximum(ends - starts, 1).astype(np.float64)
    recip = np.where(valid & (lens > 0), 1.0 / denom, 0.0).astype(np.float32)
    return starts, lens, recip, po


def _numpy_fallback(feats, pos_table, word_lens, pos, seq_len):
    feats = np.asarray(feats, np.float32)
    pos_table = np.asarray(pos_table, np.float32)
    starts, lens, recip, po = _word_ranges(word_lens, pos, seq_len)
    out = np.zeros((feats.shape[0], po.shape[1], feats.shape[2]), np.float32)
    for b in range(out.shape[0]):
        for w in range(out.shape[1]):
            L = int(lens[b, w])
            if L > 0:
                s = int(starts[b, w])
                out[b, w] = feats[b, s:s + L].sum(axis=0) * recip[b, w]
        out[b] += pos_table[po[b]]
    return out


def _concourse_importable():
    try:
        import concourse.bass  # noqa: F401
        return True
    except ImportError:
        import sys
        for p in ("/opt/trn_rl_repo", "/root/.axon_site/_ro/trn_rl_repo"):
            if p not in sys.path:
                sys.path.append(p)
        try:
            import concourse.bass  # noqa: F401
            return True
        except ImportError:
            return False


def _prepare(feats, pos_table_np, starts, lens, recip, po, kmax):
    """Host-side layout.

    Returns (geom, in_maps, meta, tot_rows):
      geom: list of (L, colbase, ncols, cu_list) in descending-L order —
            the shared program shape.
      meta[core]: extraction records (bg, word_idx_array, out_row_start).
    """
    perms = np.zeros((B, W), np.int64)
    for b in range(B):
        perms[b] = np.argsort(-lens[b], kind="stable")
    # per (core, L): words bl-major in sorted order
    SL = np.zeros((N_CORES, kmax + 1), np.int64)
    for c in range(N_CORES):
        for L in range(1, kmax + 1):
            SL[c, L] = int((lens[c * BPC:(c + 1) * BPC] == L).sum())
    MS = SL.max(axis=0)                      # unified capacity per L

    geom = []
    colbase = 0
    rowbase = 0
    for L in range(kmax, 0, -1):
        if MS[L] == 0:
            continue
        ms = int(MS[L])
        ncols = -(-ms // P)
        cu_list = [min(P, ms - c * P) for c in range(ncols)]
        geom.append((L, colbase, ncols, cu_list, rowbase, ms))
        colbase += ncols
        rowbase += ms
    ncol_total = colbase
    tot_rows = rowbase

    ident = np.eye(P, dtype=np.float16)
    in_maps = []
    meta = []
    for core in range(N_CORES):
        bs = slice(core * BPC, (core + 1) * BPC)
        feats_h = feats[bs].reshape(-1, D).astype(np.float16)
        # int16 wrapped gather indices: per L a [128, 8*ncols] block where
        # element (p, c) = flat[c*16 + p%16] (16-wrapped, replicated x8 cores)
        idx_all = np.full((P, 8 * ncol_total), -1, np.int16)
        recip_all = np.zeros((P, ncol_total), np.float32)
        oh_f = np.zeros((PV, ncol_total * P), np.float32)
        recs = []
        for (L, cb, ncols, cu_list, rb, ms) in geom:
            cap = P * ncols
            # slots [0, ms): valid rows (pads spread over low batch-0 rows);
            # slots [ms, cap): -1, never reached (num_idxs_reg = ms)
            flat = np.full(cap, -1, np.int64)
            flat[:ms] = (np.arange(ms) * 53) % (S - KMAX_DEVICE)
            slot = 0                         # slot index within this L block
            for bl in range(BPC):
                bg = core * BPC + bl
                perm = perms[bg]
                lsort = lens[bg][perm]
                gstart = int(np.searchsorted(-lsort, -L, side="left"))
                n_here = int((lens[bg] == L).sum())
                if n_here == 0:
                    continue
                wsel = perm[gstart:gstart + n_here]
                sl_idx = slot + np.arange(n_here)
                flat[sl_idx] = bl * S + starts[bg][wsel]
                pcol = sl_idx % P
                ccol = sl_idx // P
                recip_all[pcol, cb + ccol] = recip[bg][wsel]
                oh_f[po[bg][wsel], (cb + ccol) * P + pcol] = 1.0
                recs.append((bg, wsel, rb + slot))
                slot += n_here
            wrapped = flat.astype(np.int16).reshape(-1, 16).T   # [16, cap/16]
            idx_all[:, 8 * cb: 8 * (cb + ncols)] = np.tile(wrapped, (8, 1))
        in_maps.append({
            "feats_h": feats_h,
            "pos_tab": pos_table_np.astype(np.float16),
            "idx_all": idx_all,
            "recip_all": recip_all,
            "oh_all": oh_f.astype(np.float16),
            "ident": ident,
        })
        meta.append(recs)
    return geom, ncol_total, in_maps, meta, tot_rows


def _build_nc(geom, ncol_total, tot_rows, kmax):
    from concourse import bass, bacc, mybir
    import concourse.tile as tile

    nrows = BPC * S
    nc = bacc.Bacc("TRN2", target_bir_lowering=False, debug=False)
    t_feats = nc.dram_tensor("feats_h", [nrows, D], mybir.dt.float16,
                             kind="ExternalInput")
    t_pos = nc.dram_tensor("pos_tab", [PV, D], mybir.dt.float16,
                           kind="ExternalInput")
    t_idx = nc.dram_tensor("idx_all", [P, 8 * ncol_total], mybir.dt.int16,
                           kind="ExternalInput")
    t_recip = nc.dram_tensor("recip_all", [P, ncol_total], mybir.dt.float32,
                             kind="ExternalInput")
    t_oh = nc.dram_tensor("oh_all", [PV, ncol_total * P], mybir.dt.float16,
                          kind="ExternalInput")
    t_ident = nc.dram_tensor("ident", [P, P], mybir.dt.float16,
                             kind="ExternalInput")
    t_out = nc.dram_tensor("out", [tot_rows, D], mybir.dt.float16,
                           kind="ExternalOutput")

    # overlapping row views: index i -> L*D consecutive elements (rows i..i+L-1)
    views = {}
    for (L, *_rest) in geom:
        views[L] = bass.AP(t_feats[:].tensor, 0,
                           [[D, nrows - L + 1], [1, L * D]])

    with tile.TileContext(nc) as tc:
        with (
            tc.tile_pool(name="const", bufs=1) as cpool,
            tc.tile_pool(name="gath", bufs=1) as gpool,
            tc.tile_pool(name="sc", bufs=4) as spool,
            tc.tile_pool(name="osb", bufs=4) as opool,
            tc.tile_pool(name="psum", bufs=4, space="PSUM") as ppool,
        ):
            idx_sb = cpool.tile([P, 8 * ncol_total], mybir.dt.int16)
            recip_sb = cpool.tile([P, ncol_total], mybir.dt.float32)
            pos_sb = cpool.tile([PV, D], mybir.dt.float16)
            oh_sb = cpool.tile([PV, ncol_total * P], mybir.dt.float16)
            id_sb = cpool.tile([P, P], mybir.dt.float16)
            nc.sync.dma_start(out=idx_sb[:], in_=t_idx[:])
            nc.sync.dma_start(out=recip_sb[:], in_=t_recip[:])
            nc.sync.dma_start(out=pos_sb[:], in_=t_pos[:])
            nc.sync.dma_start(out=oh_sb[:], in_=t_oh[:])
            nc.sync.dma_start(out=id_sb[:], in_=t_ident[:])

            gts = {}
            for (L, cb, ncols, cu_list, rb, ms) in geom:
                gt = gpool.tile([P, ncols, L * D], mybir.dt.float16,
                                tag=f"g{L}")
                gts[L] = gt
                nc.gpsimd.dma_gather(
                    gt[:, :, :],
                    views[L],
                    idx_sb[:, 8 * cb: 8 * (cb + ncols)],
                    P * ncols,            # num_idxs (idx-buffer capacity)
                    ms,                   # num_idxs_reg (descriptors emitted)
                    L * D,
                    elem_step=D,
                    single_packet=False,
                )

            # flat column list in processing order
            colwork = []
            for (L, cb, ncols, cu_list, rb, ms) in geom:
                rowoff = rb
                for c in range(ncols):
                    colwork.append((L, cb + c, c, cu_list[c], rowoff))
                    rowoff += cu_list[c]

            pending = []          # (psum, cu, rowoff) awaiting drain, lag 2

            def drain_one():
                psum, cu, rowoff = pending.pop(0)
                osb = opool.tile([P, D], mybir.dt.float16, tag="osb")
                nc.scalar.activation(out=osb[0:cu, :], in_=psum[0:cu, :],
                                     func=mybir.ActivationFunctionType.Copy)
                nc.sync.dma_start(out=t_out[rowoff:rowoff + cu, :],
                                  in_=osb[0:cu, :])

            for (L, k, c, cu, rowoff) in colwork:
                gt = gts[L]

                def row(r):
                    return gt[0:cu, c, r * D:(r + 1) * D]

                step = 1           # pairwise in-place fold: result in row 0
                while step < L:
                    for i in range(0, L - step, 2 * step):
                        nc.vector.tensor_add(out=row(i), in0=row(i),
                                             in1=row(i + step))
                    step *= 2
                sc = spool.tile([P, D], mybir.dt.float16, tag="sc")
                nc.scalar.activation(out=sc[0:cu, :], in_=row(0),
                                     func=mybir.ActivationFunctionType.Copy,
                                     scale=recip_sb[0:cu, k:k + 1])
                psum = ppool.tile([P, D], mybir.dt.float32, space="PSUM",
                                  tag="ps")
                lhs = oh_sb[:, k * P:k * P + cu]
                nc.tensor.matmul(out=psum[0:cu, 0:512], lhsT=lhs,
                                 rhs=pos_sb[:, 0:512], start=True, stop=False)
                nc.tensor.matmul(out=psum[0:cu, 0:512],
                                 lhsT=id_sb[0:cu, 0:cu],
                                 rhs=sc[0:cu, 0:512], start=False, stop=True)
                nc.tensor.matmul(out=psum[0:cu, 512:D], lhsT=lhs,
                                 rhs=pos_sb[:, 512:D], start=True, stop=False)
                nc.tensor.matmul(out=psum[0:cu, 512:D],
                                 lhsT=id_sb[0:cu, 0:cu],
                                 rhs=sc[0:cu, 512:D], start=False, stop=True)
                pending.append((psum, cu, rowoff))
                if len(pending) > 2:
                    drain_one()
            while pending:
                drain_one()
    nc.finalize()
    return nc


def kernel(feats, pos_table, word_lens, pos, seq_len):
    global LAST_RESULTS
    feats = np.ascontiguousarray(np.asarray(feats, np.float32))
    pos_table_np = np.ascontiguousarray(np.asarray(pos_table, np.float32))
    starts, lens, recip, po = _word_ranges(word_lens, pos, seq_len)

    kmax = int(lens.max())
    shapes_ok = (
        feats.shape == (B, S, D)
        and pos_table_np.shape == (PV, D)
        and po.shape == (B, W)
        and starts.shape == (B, W)
        and np.asarray(seq_len).shape == (B,)
        and int(po.max()) < PV and int(po.min()) >= 0
    )
    if kmax > KMAX_DEVICE or kmax < 1 or not shapes_ok \
            or not _concourse_importable():
        return _numpy_fallback(feats, pos_table, word_lens, pos, seq_len)

    geom, ncol_total, in_maps, meta, tot_rows = _prepare(
        feats, pos_table_np, starts, lens, recip, po, kmax)
    nc = _build_nc(geom, ncol_total, tot_rows, kmax)

    res = _run_spmd(nc, in_maps, list(range(N_CORES)))
    LAST_RESULTS = res

    out = np.zeros((B, W, D), np.float32)
    for core in range(N_CORES):
        arr = res.results[core]["out"]            # [tot_rows, D]
        for bg, wsel, rowstart in meta[core]:
            out[bg][wsel] = arr[rowstart:rowstart + len(wsel)]
    # slots the device never computes: invalid words and len-0 words get
    # means == 0, so the exact answer is just the pos embedding row
    zmask = lens == 0
    if zmask.any():
        out[zmask] = pos_table_np[po[zmask]]
    return out


# revision 27
# speedup vs baseline: 1.5523x; 1.0144x over previous
"""TRN2 Bass kernel for nn_CharModel (segment-mean over char ranges + pos embedding).

Strategy (pure data-parallel over batch, 8 cores x 4 batches):
  - Words are contiguous char ranges [start, start+L). Host sorts each batch's
    words by length L desc; same-length words across the core's 4 batches are
    packed (bl-major) into 128-partition columns. Per length L ONE
    indirect_dma_start call gathers every word's L rows as a single contiguous
    descriptor (L*768 bf16 elements via an overlapping row view) into
    [128, ncols_L, L*768] -- ~2.7k descriptors per core on the library-free
    INDIRECT1D q7 path, full-width so they spread evenly over all 16 SDMA
    engines.
  - feats is cast to bf16 on the host, halving HBM read traffic; sums
    accumulate in fp32 on DVE (L-1 adds per column), so only the per-element
    bf16 input rounding (~2^-9 relative) is lost.
  - SPMD runs one program on 8 cores: only the column COUNT per L is unified
    (max over cores); which word sits in which slot is per-core input data.
    Pad slots gather real rows (cheap, valid) and are neutralized by recip=0
    and a zero one-hot column; their output rows are discarded on the host.
  - Pos embedding via a host-built one-hot bf16 matmul on PE (PSUM), fused
    with the 1/len scaling in one scalar_tensor_tensor per column:
       out = (acc * recip) + psum_pos
  - Each column's finished [cu, 768] fp32 block DMAs straight to its row
    range of a flat output tensor; host scatters rows back to (batch, word)
    order. Rows the device never computes (len-0/invalid words) are exactly
    the pos-embedding row, filled on the host from the fp32 table.
"""

import numpy as np

B, S, W, D, PV = 32, 2048, 512, 768, 64
N_CORES = 8
BPC = B // N_CORES          # batches per core
P = 128
KMAX_DEVICE = 16            # device path supports word len up to this

LAST_RESULTS = None         # BassKernelResults of the most recent run (for test.py)


def _run_spmd(nc, in_maps, core_ids):
    """Indirection point so tests can swap in a simulator."""
    from concourse.bass_utils import run_bass_kernel_spmd
    return run_bass_kernel_spmd(nc, in_maps, core_ids)


def _word_ranges(word_lens, pos, seq_len):
    """Replicate the reference's starts/ends/valid computation in numpy."""
    wl = np.asarray(word_lens, np.int64)
    po = np.asarray(pos, np.int64)
    sl = np.asarray(seq_len, np.int64)
    b, w = wl.shape
    j = np.arange(w)
    next_start = np.concatenate([wl[:, 1:], np.zeros((b, 1), np.int64)], axis=1)
    is_last = (j[None, :] == w - 1) | (next_start == 0)
    starts = wl
    ends = np.where(is_last, sl[:, None], next_start)
    valid = (wl != 0) | (j[None, :] == 0)
    lens = np.where(valid, np.maximum(ends - starts, 0), 0)
    denom = np.maximum(ends - starts, 1).astype(np.float64)
    recip = np.where(valid & (lens > 0), 1.0 / denom, 0.0).astype(np.float32)
    return starts, lens, recip, po


def _numpy_fallback(feats, pos_table, word_lens, pos, seq_len):
    feats = np.asarray(feats, np.float32)
    pos_table = np.asarray(pos_table, np.float32)
    starts, lens, recip, po = _word_ranges(word_lens, pos, seq_len)
    out = np.zeros((feats.shape[0], po.shape[1], feats.shape[2]), np.float32)
    for b in range(out.shape[0]):
        for w in range(out.shape[1]):
            L = int(lens[b, w])
            if L > 0:
                s = int(starts[b, w])
                out[b, w] = feats[b, s:s + L].sum(axis=0) * recip[b, w]
        out[b] += pos_table[po[b]]
    return out


def _concourse_importable():
    try:
        import concourse.bass  # noqa: F401
        return True
    except ImportError:
        import sys
        for p in ("/opt/trn_rl_repo", "/root/.axon_site/_ro/trn_rl_repo"):
            if p not in sys.path:
                sys.path.append(p)
        try:
            import concourse.bass  # noqa: F401
            return True
        except ImportError:
            return False


def _prepare(feats, pos_table_np, starts, lens, recip, po, kmax):
    """Host-side layout.

    Returns (geom, in_maps, meta, tot_rows):
      geom: list of (L, colbase, ncols, cu_list) in descending-L order —
            the shared program shape.
      meta[core]: extraction records (bg, word_idx_array, out_row_start).
    """
    perms = np.zeros((B, W), np.int64)
    for b in range(B):
        perms[b] = np.argsort(-lens[b], kind="stable")
    # per (core, L): words bl-major in sorted order
    SL = np.zeros((N_CORES, kmax + 1), np.int64)
    for c in range(N_CORES):
        for L in range(1, kmax + 1):
            SL[c, L] = int((lens[c * BPC:(c + 1) * BPC] == L).sum())
    MS = SL.max(axis=0)                      # unified capacity per L

    geom = []
    colbase = 0
    rowbase = 0
    # smallest gather first (fast pipeline ramp), small group last (short
    # tail); the big-L groups stream in the middle
    order = [1] + list(range(kmax, 1, -1))
    for L in order:
        if L > kmax or MS[L] == 0:
            continue
        ms = int(MS[L])
        ncols = -(-ms // P)
        cu_list = [min(P, ms - c * P) for c in range(ncols)]
        geom.append((L, colbase, ncols, cu_list, rowbase, ms))
        colbase += ncols
        rowbase += ms
    ncol_total = colbase
    tot_rows = rowbase

    in_maps = []
    meta = []
    for core in range(N_CORES):
        bs = slice(core * BPC, (core + 1) * BPC)
        feats_h = feats[bs].reshape(-1, D).astype(np.float16)
        # int16 wrapped gather indices: per L a [128, 8*ncols] block where
        # element (p, c) = flat[c*16 + p%16] (16-wrapped, replicated x8 cores)
        idx_all = np.full((P, 8 * ncol_total), -1, np.int16)
        recd = np.zeros((P, ncol_total * P), np.float16)   # diag(recip)/column
        oh_f = np.zeros((PV, ncol_total * P), np.float32)
        recs = []
        for (L, cb, ncols, cu_list, rb, ms) in geom:
            cap = P * ncols
            # slots [0, ms): valid rows (pads spread over low batch-0 rows);
            # slots [ms, cap): -1, never reached (num_idxs_reg = ms)
            flat = np.full(cap, -1, np.int64)
            flat[:ms] = (np.arange(ms) * 53) % (S - KMAX_DEVICE)
            slot = 0                         # slot index within this L block
            for bl in range(BPC):
                bg = core * BPC + bl
                perm = perms[bg]
                lsort = lens[bg][perm]
                gstart = int(np.searchsorted(-lsort, -L, side="left"))
                n_here = int((lens[bg] == L).sum())
                if n_here == 0:
                    continue
                wsel = perm[gstart:gstart + n_here]
                sl_idx = slot + np.arange(n_here)
                flat[sl_idx] = bl * S + starts[bg][wsel]
                pcol = sl_idx % P
                ccol = sl_idx // P
                recd[pcol, (cb + ccol) * P + pcol] = recip[bg][wsel]
                oh_f[po[bg][wsel], (cb + ccol) * P + pcol] = 1.0
                recs.append((bg, wsel, rb + slot))
                slot += n_here
            wrapped = flat.astype(np.int16).reshape(-1, 16).T   # [16, cap/16]
            idx_all[:, 8 * cb: 8 * (cb + ncols)] = np.tile(wrapped, (8, 1))
        in_maps.append({
            "feats_h": feats_h,
            "pos_tab": pos_table_np.astype(np.float16),
            "idx_all": idx_all,
            "recd": recd,
            "oh_all": oh_f.astype(np.float16),
        })
        meta.append(recs)
    return geom, ncol_total, in_maps, meta, tot_rows


def _build_nc(geom, ncol_total, tot_rows, kmax):
    from concourse import bass, bacc, mybir
    import concourse.tile as tile

    nrows = BPC * S
    nc = bacc.Bacc("TRN2", target_bir_lowering=False, debug=False)
    t_feats = nc.dram_tensor("feats_h", [nrows, D], mybir.dt.float16,
                             kind="ExternalInput")
    t_pos = nc.dram_tensor("pos_tab", [PV, D], mybir.dt.float16,
                           kind="ExternalInput")
    t_idx = nc.dram_tensor("idx_all", [P, 8 * ncol_total], mybir.dt.int16,
                           kind="ExternalInput")
    t_recd = nc.dram_tensor("recd", [P, ncol_total * P], mybir.dt.float16,
                            kind="ExternalInput")
    t_oh = nc.dram_tensor("oh_all", [PV, ncol_total * P], mybir.dt.float16,
                          kind="ExternalInput")
    t_out = nc.dram_tensor("out", [tot_rows, D], mybir.dt.float16,
                           kind="ExternalOutput")

    # overlapping row views: index i -> L*D consecutive elements (rows i..i+L-1)
    views = {}
    for (L, *_rest) in geom:
        views[L] = bass.AP(t_feats[:].tensor, 0,
                           [[D, nrows - L + 1], [1, L * D]])

    with tile.TileContext(nc) as tc:
        with (
            tc.tile_pool(name="const", bufs=1) as cpool,
            tc.tile_pool(name="gath", bufs=1) as gpool,
            tc.tile_pool(name="osb", bufs=4) as opool,
            tc.tile_pool(name="psum", bufs=4, space="PSUM") as ppool,
        ):
            idx_sb = cpool.tile([P, 8 * ncol_total], mybir.dt.int16)
            recd_sb = cpool.tile([P, ncol_total * P], mybir.dt.float16)
            pos_sb = cpool.tile([PV, D], mybir.dt.float16)
            oh_sb = cpool.tile([PV, ncol_total * P], mybir.dt.float16)
            nc.sync.dma_start(out=idx_sb[:], in_=t_idx[:])
            nc.sync.dma_start(out=pos_sb[:], in_=t_pos[:])
            nc.sync.dma_start(out=oh_sb[:], in_=t_oh[:])
            nc.sync.dma_start(out=recd_sb[:], in_=t_recd[:])

            gts = {}
            for (L, cb, ncols, cu_list, rb, ms) in geom:
                gt = gpool.tile([P, ncols, L * D], mybir.dt.float16,
                                tag=f"g{L}")
                gts[L] = gt
                nc.gpsimd.dma_gather(
                    gt[:, :, :],
                    views[L],
                    idx_sb[:, 8 * cb: 8 * (cb + ncols)],
                    P * ncols,            # num_idxs (idx-buffer capacity)
                    ms,                   # num_idxs_reg (descriptors emitted)
                    L * D,
                    elem_step=D,
                    single_packet=False,
                )

            # flat column list in processing order
            colwork = []
            for (L, cb, ncols, cu_list, rb, ms) in geom:
                rowoff = rb
                for c in range(ncols):
                    colwork.append((L, cb + c, c, cu_list[c], rowoff))
                    rowoff += cu_list[c]

            pending = []          # (psum, cu, rowoff) awaiting drain, lag 2

            def drain_one():
                psum, cu, rowoff = pending.pop(0)
                osb = opool.tile([P, D], mybir.dt.float16, tag="osb")
                nc.scalar.activation(out=osb[0:cu, :], in_=psum[0:cu, :],
                                     func=mybir.ActivationFunctionType.Copy)
                nc.sync.dma_start(out=t_out[rowoff:rowoff + cu, :],
                                  in_=osb[0:cu, :])

            for (L, k, c, cu, rowoff) in colwork:
                gt = gts[L]

                def row(r):
                    return gt[0:cu, c, r * D:(r + 1) * D]

                step = 1           # pairwise in-place fold: result in row 0
                while step < L:
                    for i in range(0, L - step, 2 * step):
                        nc.vector.tensor_add(out=row(i), in0=row(i),
                                             in1=row(i + step))
                    step *= 2
                psum = ppool.tile([P, D], mybir.dt.float32, space="PSUM",
                                  tag="ps")
                lhs = oh_sb[:, k * P:k * P + cu]
                dg = recd_sb[0:cu, k * P:k * P + cu]
                nc.tensor.matmul(out=psum[0:cu, 0:512], lhsT=lhs,
                                 rhs=pos_sb[:, 0:512], start=True, stop=False)
                nc.tensor.matmul(out=psum[0:cu, 0:512], lhsT=dg,
                                 rhs=gt[0:cu, c, 0:512], start=False,
                                 stop=True)
                nc.tensor.matmul(out=psum[0:cu, 512:D], lhsT=lhs,
                                 rhs=pos_sb[:, 512:D], start=True, stop=False)
                nc.tensor.matmul(out=psum[0:cu, 512:D], lhsT=dg,
                                 rhs=gt[0:cu, c, 512:D], start=False,
                                 stop=True)
                pending.append((psum, cu, rowoff))
                if len(pending) > 2:
                    drain_one()
            while pending:
                drain_one()
    nc.finalize()
    return nc


def kernel(feats, pos_table, word_lens, pos, seq_len):
    global LAST_RESULTS
    feats = np.ascontiguousarray(np.asarray(feats, np.float32))
    pos_table_np = np.ascontiguousarray(np.asarray(pos_table, np.float32))
    starts, lens, recip, po = _word_ranges(word_lens, pos, seq_len)

    kmax = int(lens.max())
    shapes_ok = (
        feats.shape == (B, S, D)
        and pos_table_np.shape == (PV, D)
        and po.shape == (B, W)
        and starts.shape == (B, W)
        and np.asarray(seq_len).shape == (B,)
        and int(po.max()) < PV and int(po.min()) >= 0
    )
    if kmax > KMAX_DEVICE or kmax < 1 or not shapes_ok \
            or not _concourse_importable():
        return _numpy_fallback(feats, pos_table, word_lens, pos, seq_len)

    geom, ncol_total, in_maps, meta, tot_rows = _prepare(
        feats, pos_table_np, starts, lens, recip, po, kmax)
    nc = _build_nc(geom, ncol_total, tot_rows, kmax)

    res = _run_spmd(nc, in_maps, list(range(N_CORES)))
    LAST_RESULTS = res

    out = np.zeros((B, W, D), np.float32)
    for core in range(N_CORES):
        arr = res.results[core]["out"]            # [tot_rows, D]
        for bg, wsel, rowstart in meta[core]:
            out[bg][wsel] = arr[rowstart:rowstart + len(wsel)]
    # slots the device never computes: invalid words and len-0 words get
    # means == 0, so the exact answer is just the pos embedding row
    zmask = lens == 0
    if zmask.any():
        out[zmask] = pos_table_np[po[zmask]]
    return out


# revision 32
# speedup vs baseline: 1.5648x; 1.0081x over previous
"""TRN2 Bass kernel for nn_CharModel (segment-mean over char ranges + pos embedding).

Strategy (pure data-parallel over batch, 8 cores x 4 batches):
  - Words are contiguous char ranges [start, start+L). Host sorts each batch's
    words by length L desc; same-length words across the core's 4 batches are
    packed (bl-major) into 128-partition columns. Per length L ONE
    indirect_dma_start call gathers every word's L rows as a single contiguous
    descriptor (L*768 bf16 elements via an overlapping row view) into
    [128, ncols_L, L*768] -- ~2.7k descriptors per core on the library-free
    INDIRECT1D q7 path, full-width so they spread evenly over all 16 SDMA
    engines.
  - feats is cast to bf16 on the host, halving HBM read traffic; sums
    accumulate in fp32 on DVE (L-1 adds per column), so only the per-element
    bf16 input rounding (~2^-9 relative) is lost.
  - SPMD runs one program on 8 cores: only the column COUNT per L is unified
    (max over cores); which word sits in which slot is per-core input data.
    Pad slots gather real rows (cheap, valid) and are neutralized by recip=0
    and a zero one-hot column; their output rows are discarded on the host.
  - Pos embedding via a host-built one-hot bf16 matmul on PE (PSUM), fused
    with the 1/len scaling in one scalar_tensor_tensor per column:
       out = (acc * recip) + psum_pos
  - Each column's finished [cu, 768] fp32 block DMAs straight to its row
    range of a flat output tensor; host scatters rows back to (batch, word)
    order. Rows the device never computes (len-0/invalid words) are exactly
    the pos-embedding row, filled on the host from the fp32 table.
"""

import numpy as np

B, S, W, D, PV = 32, 2048, 512, 768, 64
N_CORES = 8
BPC = B // N_CORES          # batches per core
P = 128
KMAX_DEVICE = 16            # device path supports word len up to this

LAST_RESULTS = None         # BassKernelResults of the most recent run (for test.py)


def _run_spmd(nc, in_maps, core_ids):
    """Indirection point so tests can swap in a simulator."""
    from concourse.bass_utils import run_bass_kernel_spmd
    return run_bass_kernel_spmd(nc, in_maps, core_ids)


def _word_ranges(word_lens, pos, seq_len):
    """Replicate the reference's starts/ends/valid computation in numpy."""
    wl = np.asarray(word_lens, np.int64)
    po = np.asarray(pos, np.int64)
    sl = np.asarray(seq_len, np.int64)
    b, w = wl.shape
    j = np.arange(w)
    next_start = np.concatenate([wl[:, 1:], np.zeros((b, 1), np.int64)], axis=1)
    is_last = (j[None, :] == w - 1) | (next_start == 0)
    starts = wl
    ends = np.where(is_last, sl[:, None], next_start)
    valid = (wl != 0) | (j[None, :] == 0)
    lens = np.where(valid, np.maximum(ends - starts, 0), 0)
    denom = np.maximum(ends - starts, 1).astype(np.float64)
    recip = np.where(valid & (lens > 0), 1.0 / denom, 0.0).astype(np.float32)
    return starts, lens, recip, po


def _numpy_fallback(feats, pos_table, word_lens, pos, seq_len):
    feats = np.asarray(feats, np.float32)
    pos_table = np.asarray(pos_table, np.float32)
    starts, lens, recip, po = _word_ranges(word_lens, pos, seq_len)
    out = np.zeros((feats.shape[0], po.shape[1], feats.shape[2]), np.float32)
    for b in range(out.shape[0]):
        for w in range(out.shape[1]):
            L = int(lens[b, w])
            if L > 0:
                s = int(starts[b, w])
                out[b, w] = feats[b, s:s + L].sum(axis=0) * recip[b, w]
        out[b] += pos_table[po[b]]
    return out


def _concourse_importable():
    try:
        import concourse.bass  # noqa: F401
        return True
    except ImportError:
        import sys
        for p in ("/opt/trn_rl_repo", "/root/.axon_site/_ro/trn_rl_repo"):
            if p not in sys.path:
                sys.path.append(p)
        try:
            import concourse.bass  # noqa: F401
            return True
        except ImportError:
            return False


def _prepare(feats, pos_table_np, starts, lens, recip, po, kmax):
    """Host-side layout.

    Returns (geom, in_maps, meta, tot_rows):
      geom: list of (L, colbase, ncols, cu_list) in descending-L order —
            the shared program shape.
      meta[core]: extraction records (bg, word_idx_array, out_row_start).
    """
    perms = np.zeros((B, W), np.int64)
    for b in range(B):
        perms[b] = np.argsort(-lens[b], kind="stable")
    # per (core, L): words bl-major in sorted order
    SL = np.zeros((N_CORES, kmax + 1), np.int64)
    for c in range(N_CORES):
        for L in range(1, kmax + 1):
            SL[c, L] = int((lens[c * BPC:(c + 1) * BPC] == L).sum())
    MS = SL.max(axis=0)                      # unified capacity per L

    geom = []
    colbase = 0
    rowbase = 0
    # smallest gather first (fast pipeline ramp), small group last (short
    # tail); the big-L groups stream in the middle
    order = [1] + list(range(kmax, 1, -1))
    for L in order:
        if L > kmax or MS[L] == 0:
            continue
        ms = int(MS[L])
        ncols = -(-ms // P)
        cu_list = [min(P, ms - c * P) for c in range(ncols)]
        geom.append((L, colbase, ncols, cu_list, rowbase, ms))
        colbase += ncols
        rowbase += ms
    ncol_total = colbase
    tot_rows = rowbase

    in_maps = []
    meta = []
    for core in range(N_CORES):
        bs = slice(core * BPC, (core + 1) * BPC)
        feats_h = feats[bs].reshape(-1, D).astype(np.float16)
        # int16 wrapped gather indices: per L a [128, 8*ncols] block where
        # element (p, c) = flat[c*16 + p%16] (16-wrapped, replicated x8 cores)
        idx_all = np.full((P, 8 * ncol_total), -1, np.int16)
        n1 = next(g[2] for g in geom if g[0] == 1)       # ncols of L=1
        idx1 = np.zeros((P, n1), np.int32)               # library-free path
        recd = np.zeros((P, ncol_total * P), np.float16)   # diag(recip)/column
        oh_f = np.zeros((PV, ncol_total * P), np.float32)
        recs = []
        for (L, cb, ncols, cu_list, rb, ms) in geom:
            cap = P * ncols
            # slots [0, ms): valid rows (pads spread over low batch-0 rows);
            # slots [ms, cap): -1, never reached (num_idxs_reg = ms)
            flat = np.full(cap, -1, np.int64)
            flat[:ms] = (np.arange(ms) * 53) % (S - KMAX_DEVICE)
            if L == 1:
                flat[ms:] = 0                            # indirect path reads all
            slot = 0                         # slot index within this L block
            for bl in range(BPC):
                bg = core * BPC + bl
                perm = perms[bg]
                lsort = lens[bg][perm]
                gstart = int(np.searchsorted(-lsort, -L, side="left"))
                n_here = int((lens[bg] == L).sum())
                if n_here == 0:
                    continue
                wsel = perm[gstart:gstart + n_here]
                sl_idx = slot + np.arange(n_here)
                flat[sl_idx] = bl * S + starts[bg][wsel]
                pcol = sl_idx % P
                ccol = sl_idx // P
                recd[pcol, (cb + ccol) * P + pcol] = recip[bg][wsel]
                oh_f[po[bg][wsel], (cb + ccol) * P + pcol] = 1.0
                recs.append((bg, wsel, rb + slot))
                slot += n_here
            if L == 1:
                idx1[:, :] = flat.reshape(ncols, P).T.astype(np.int32)
            else:
                wrapped = flat.astype(np.int16).reshape(-1, 16).T  # [16, cap/16]
                idx_all[:, 8 * cb: 8 * (cb + ncols)] = np.tile(wrapped, (8, 1))
        in_maps.append({
            "feats_h": feats_h,
            "pos_tab": pos_table_np.astype(np.float16),
            "idx_all": idx_all,
            "idx1": idx1,
            "recd": recd,
            "oh_all": oh_f.astype(np.float16),
        })
        meta.append(recs)
    return geom, ncol_total, in_maps, meta, tot_rows


def _build_nc(geom, ncol_total, tot_rows, kmax):
    from concourse import bass, bacc, mybir
    import concourse.tile as tile

    nrows = BPC * S
    nc = bacc.Bacc("TRN2", target_bir_lowering=False, debug=False)
    t_feats = nc.dram_tensor("feats_h", [nrows, D], mybir.dt.float16,
                             kind="ExternalInput")
    t_pos = nc.dram_tensor("pos_tab", [PV, D], mybir.dt.float16,
                           kind="ExternalInput")
    t_idx = nc.dram_tensor("idx_all", [P, 8 * ncol_total], mybir.dt.int16,
                           kind="ExternalInput")
    n1 = next(g[2] for g in geom if g[0] == 1)
    t_idx1 = nc.dram_tensor("idx1", [P, n1], mybir.dt.int32,
                            kind="ExternalInput")
    t_recd = nc.dram_tensor("recd", [P, ncol_total * P], mybir.dt.float16,
                            kind="ExternalInput")
    t_oh = nc.dram_tensor("oh_all", [PV, ncol_total * P], mybir.dt.float16,
                          kind="ExternalInput")
    t_out = nc.dram_tensor("out", [tot_rows, D], mybir.dt.float16,
                           kind="ExternalOutput")

    # overlapping row views: index i -> L*D consecutive elements (rows i..i+L-1)
    views = {}
    for (L, *_rest) in geom:
        views[L] = bass.AP(t_feats[:].tensor, 0,
                           [[D, nrows - L + 1], [1, L * D]])

    with tile.TileContext(nc) as tc:
        with (
            tc.tile_pool(name="const", bufs=1) as cpool,
            tc.tile_pool(name="gath", bufs=1) as gpool,
            tc.tile_pool(name="osb", bufs=4) as opool,
            tc.tile_pool(name="psum", bufs=4, space="PSUM") as ppool,
        ):
            idx1_sb = cpool.tile([P, n1], mybir.dt.int32)
            idx_sb = cpool.tile([P, 8 * ncol_total], mybir.dt.int16)
            recd_sb = cpool.tile([P, ncol_total * P], mybir.dt.float16)
            pos_sb = cpool.tile([PV, D], mybir.dt.float16)
            oh_sb = cpool.tile([PV, ncol_total * P], mybir.dt.float16)
            nc.sync.dma_start(out=idx1_sb[:], in_=t_idx1[:])
            nc.sync.dma_start(out=idx_sb[:], in_=t_idx[:])
            nc.sync.dma_start(out=pos_sb[:], in_=t_pos[:])
            nc.sync.dma_start(out=oh_sb[:], in_=t_oh[:])
            nc.sync.dma_start(out=recd_sb[:], in_=t_recd[:])

            gts = {}
            # L=1 first, on the library-free INDIRECT1D path: its descriptors
            # generate while the q7 dma_gather library is still being fetched
            for (L, cb, ncols, cu_list, rb, ms) in geom:
                if L != 1:
                    continue
                gt = gpool.tile([P, ncols, L * D], mybir.dt.float16,
                                tag=f"g{L}")
                gts[L] = gt
                for c in range(ncols):
                    nc.gpsimd.indirect_dma_start(
                        out=gt[:, c, :],
                        out_offset=None,
                        in_=t_feats[:],
                        in_offset=bass.IndirectOffsetOnAxis(
                            ap=idx1_sb[:, c:c + 1], axis=0),
                    )
            for (L, cb, ncols, cu_list, rb, ms) in geom:
                if L == 1:
                    continue
                gt = gpool.tile([P, ncols, L * D], mybir.dt.float16,
                                tag=f"g{L}")
                gts[L] = gt
                nc.gpsimd.dma_gather(
                    gt[:, :, :],
                    views[L],
                    idx_sb[:, 8 * cb: 8 * (cb + ncols)],
                    P * ncols,            # num_idxs (idx-buffer capacity)
                    ms,                   # num_idxs_reg (descriptors emitted)
                    L * D,
                    elem_step=D,
                    single_packet=False,
                )

            # flat column list in processing order
            colwork = []
            for (L, cb, ncols, cu_list, rb, ms) in geom:
                rowoff = rb
                for c in range(ncols):
                    colwork.append((L, cb + c, c, cu_list[c], rowoff))
                    rowoff += cu_list[c]

            pending = []          # (psum, cu, rowoff) awaiting drain, lag 2

            def drain_one():
                psum, cu, rowoff = pending.pop(0)
                osb = opool.tile([P, D], mybir.dt.float16, tag="osb")
                nc.scalar.activation(out=osb[0:cu, :], in_=psum[0:cu, :],
                                     func=mybir.ActivationFunctionType.Copy)
                nc.sync.dma_start(out=t_out[rowoff:rowoff + cu, :],
                                  in_=osb[0:cu, :])

            for (L, k, c, cu, rowoff) in colwork:
                gt = gts[L]

                def row(r):
                    return gt[0:cu, c, r * D:(r + 1) * D]

                step = 1           # pairwise in-place fold: result in row 0
                while step < L:
                    for i in range(0, L - step, 2 * step):
                        nc.vector.tensor_add(out=row(i), in0=row(i),
                                             in1=row(i + step))
                    step *= 2
                psum = ppool.tile([P, D], mybir.dt.float32, space="PSUM",
                                  tag="ps")
                lhs = oh_sb[:, k * P:k * P + cu]
                dg = recd_sb[0:cu, k * P:k * P + cu]
                nc.tensor.matmul(out=psum[0:cu, 0:512], lhsT=lhs,
                                 rhs=pos_sb[:, 0:512], start=True, stop=False)
                nc.tensor.matmul(out=psum[0:cu, 0:512], lhsT=dg,
                                 rhs=gt[0:cu, c, 0:512], start=False,
                                 stop=True)
                nc.tensor.matmul(out=psum[0:cu, 512:D], lhsT=lhs,
                                 rhs=pos_sb[:, 512:D], start=True, stop=False)
                nc.tensor.matmul(out=psum[0:cu, 512:D], lhsT=dg,
                                 rhs=gt[0:cu, c, 512:D], start=False,
                                 stop=True)
                pending.append((psum, cu, rowoff))
                if len(pending) > 1:
                    drain_one()
            while pending:
                drain_one()
    nc.finalize()
    return nc


def kernel(feats, pos_table, word_lens, pos, seq_len):
    global LAST_RESULTS
    feats = np.ascontiguousarray(np.asarray(feats, np.float32))
    pos_table_np = np.ascontiguousarray(np.asarray(pos_table, np.float32))
    starts, lens, recip, po = _word_ranges(word_lens, pos, seq_len)

    kmax = int(lens.max())
    shapes_ok = (
        feats.shape == (B, S, D)
        and pos_table_np.shape == (PV, D)
        and po.shape == (B, W)
        and starts.shape == (B, W)
        and np.asarray(seq_len).shape == (B,)
        and int(po.max()) < PV and int(po.min()) >= 0
    )
    if kmax > KMAX_DEVICE or kmax < 1 or not shapes_ok \
            or not _concourse_importable():
        return _numpy_fallback(feats, pos_table, word_lens, pos, seq_len)

    geom, ncol_total, in_maps, meta, tot_rows = _prepare(
        feats, pos_table_np, starts, lens, recip, po, kmax)
    nc = _build_nc(geom, ncol_total, tot_rows, kmax)

    res = _run_spmd(nc, in_maps, list(range(N_CORES)))
    LAST_RESULTS = res

    out = np.zeros((B, W, D), np.float32)
    for core in range(N_CORES):
        arr = res.results[core]["out"]            # [tot_rows, D]
        for bg, wsel, rowstart in meta[core]:
            out[bg][wsel] = arr[rowstart:rowstart + len(wsel)]
    # slots the device never computes: invalid words and len-0 words get
    # means == 0, so the exact answer is just the pos embedding row
    zmask = lens == 0
    if zmask.any():
        out[zmask] = pos_table_np[po[zmask]]
    return out


# revision 34
# speedup vs baseline: 1.5728x; 1.0051x over previous
"""TRN2 Bass kernel for nn_CharModel (segment-mean over char ranges + pos embedding).

Strategy (pure data-parallel over batch, 8 cores x 4 batches):
  - Words are contiguous char ranges [start, start+L). Host sorts each batch's
    words by length L desc; same-length words across the core's 4 batches are
    packed (bl-major) into 128-partition columns. Per length L ONE
    indirect_dma_start call gathers every word's L rows as a single contiguous
    descriptor (L*768 bf16 elements via an overlapping row view) into
    [128, ncols_L, L*768] -- ~2.7k descriptors per core on the library-free
    INDIRECT1D q7 path, full-width so they spread evenly over all 16 SDMA
    engines.
  - feats is cast to bf16 on the host, halving HBM read traffic; sums
    accumulate in fp32 on DVE (L-1 adds per column), so only the per-element
    bf16 input rounding (~2^-9 relative) is lost.
  - SPMD runs one program on 8 cores: only the column COUNT per L is unified
    (max over cores); which word sits in which slot is per-core input data.
    Pad slots gather real rows (cheap, valid) and are neutralized by recip=0
    and a zero one-hot column; their output rows are discarded on the host.
  - Pos embedding via a host-built one-hot bf16 matmul on PE (PSUM), fused
    with the 1/len scaling in one scalar_tensor_tensor per column:
       out = (acc * recip) + psum_pos
  - Each column's finished [cu, 768] fp32 block DMAs straight to its row
    range of a flat output tensor; host scatters rows back to (batch, word)
    order. Rows the device never computes (len-0/invalid words) are exactly
    the pos-embedding row, filled on the host from the fp32 table.
"""

import numpy as np

B, S, W, D, PV = 32, 2048, 512, 768, 64
N_CORES = 8
BPC = B // N_CORES          # batches per core
P = 128
KMAX_DEVICE = 16            # device path supports word len up to this

LAST_RESULTS = None         # BassKernelResults of the most recent run (for test.py)


def _run_spmd(nc, in_maps, core_ids):
    """Indirection point so tests can swap in a simulator."""
    from concourse.bass_utils import run_bass_kernel_spmd
    return run_bass_kernel_spmd(nc, in_maps, core_ids)


def _word_ranges(word_lens, pos, seq_len):
    """Replicate the reference's starts/ends/valid computation in numpy."""
    wl = np.asarray(word_lens, np.int64)
    po = np.asarray(pos, np.int64)
    sl = np.asarray(seq_len, np.int64)
    b, w = wl.shape
    j = np.arange(w)
    next_start = np.concatenate([wl[:, 1:], np.zeros((b, 1), np.int64)], axis=1)
    is_last = (j[None, :] == w - 1) | (next_start == 0)
    starts = wl
    ends = np.where(is_last, sl[:, None], next_start)
    valid = (wl != 0) | (j[None, :] == 0)
    lens = np.where(valid, np.maximum(ends - starts, 0), 0)
    denom = np.maximum(ends - starts, 1).astype(np.float64)
    recip = np.where(valid & (lens > 0), 1.0 / denom, 0.0).astype(np.float32)
    return starts, lens, recip, po


def _numpy_fallback(feats, pos_table, word_lens, pos, seq_len):
    feats = np.asarray(feats, np.float32)
    pos_table = np.asarray(pos_table, np.float32)
    starts, lens, recip, po = _word_ranges(word_lens, pos, seq_len)
    out = np.zeros((feats.shape[0], po.shape[1], feats.shape[2]), np.float32)
    for b in range(out.shape[0]):
        for w in range(out.shape[1]):
            L = int(lens[b, w])
            if L > 0:
                s = int(starts[b, w])
                out[b, w] = feats[b, s:s + L].sum(axis=0) * recip[b, w]
        out[b] += pos_table[po[b]]
    return out


def _concourse_importable():
    try:
        import concourse.bass  # noqa: F401
        return True
    except ImportError:
        import sys
        for p in ("/opt/trn_rl_repo", "/root/.axon_site/_ro/trn_rl_repo"):
            if p not in sys.path:
                sys.path.append(p)
        try:
            import concourse.bass  # noqa: F401
            return True
        except ImportError:
            return False


def _prepare(feats, pos_table_np, starts, lens, recip, po, kmax):
    """Host-side layout.

    Returns (geom, in_maps, meta, tot_rows):
      geom: list of (L, colbase, ncols, cu_list) in descending-L order —
            the shared program shape.
      meta[core]: extraction records (bg, word_idx_array, out_row_start).
    """
    perms = np.zeros((B, W), np.int64)
    for b in range(B):
        perms[b] = np.argsort(-lens[b], kind="stable")
    # per (core, L): words bl-major in sorted order
    SL = np.zeros((N_CORES, kmax + 1), np.int64)
    for c in range(N_CORES):
        for L in range(1, kmax + 1):
            SL[c, L] = int((lens[c * BPC:(c + 1) * BPC] == L).sum())
    MS = SL.max(axis=0)                      # unified capacity per L

    geom = []
    colbase = 0
    rowbase = 0
    # smallest gather first (fast pipeline ramp), small group last (short
    # tail); the big-L groups stream in the middle
    order = [1] + list(range(kmax, 1, -1))
    for L in order:
        if L > kmax or MS[L] == 0:
            continue
        ms = int(MS[L])
        ncols = -(-ms // P)
        cu_list = [min(P, ms - c * P) for c in range(ncols)]
        geom.append((L, colbase, ncols, cu_list, rowbase, ms))
        colbase += ncols
        rowbase += ms
    ncol_total = colbase
    tot_rows = rowbase

    in_maps = []
    meta = []
    for core in range(N_CORES):
        bs = slice(core * BPC, (core + 1) * BPC)
        feats_h = feats[bs].reshape(-1, D).astype(np.float16)
        # int16 wrapped gather indices: per L a [128, 8*ncols] block where
        # element (p, c) = flat[c*16 + p%16] (16-wrapped, replicated x8 cores)
        idx_all = np.full((P, 8 * ncol_total), -1, np.int16)
        n1 = next(g[2] for g in geom if g[0] == 1)       # ncols of L=1
        idx1 = np.zeros((P, n1), np.int32)               # library-free path
        recd = np.zeros((P, ncol_total * P), np.float16)   # diag(recip)/column
        oh_f = np.zeros((PV, ncol_total * P), np.float32)
        recs = []
        for (L, cb, ncols, cu_list, rb, ms) in geom:
            cap = P * ncols
            # slots [0, ms): valid rows (pads spread over low batch-0 rows);
            # slots [ms, cap): -1, never reached (num_idxs_reg = ms)
            flat = np.full(cap, -1, np.int64)
            flat[:ms] = (np.arange(ms) * 53) % (S - KMAX_DEVICE)
            if L == 1:
                flat[ms:] = 0                            # indirect path reads all
            slot = 0                         # slot index within this L block
            for bl in range(BPC):
                bg = core * BPC + bl
                perm = perms[bg]
                lsort = lens[bg][perm]
                gstart = int(np.searchsorted(-lsort, -L, side="left"))
                n_here = int((lens[bg] == L).sum())
                if n_here == 0:
                    continue
                wsel = perm[gstart:gstart + n_here]
                sl_idx = slot + np.arange(n_here)
                flat[sl_idx] = bl * S + starts[bg][wsel]
                pcol = sl_idx % P
                ccol = sl_idx // P
                recd[pcol, (cb + ccol) * P + pcol] = recip[bg][wsel]
                oh_f[po[bg][wsel], (cb + ccol) * P + pcol] = 1.0
                recs.append((bg, wsel, rb + slot))
                slot += n_here
            if L == 1:
                idx1[:, :] = flat.reshape(ncols, P).T.astype(np.int32)
            else:
                wrapped = flat.astype(np.int16).reshape(-1, 16).T  # [16, cap/16]
                idx_all[:, 8 * cb: 8 * (cb + ncols)] = np.tile(wrapped, (8, 1))
        in_maps.append({
            "feats_h": feats_h,
            "pos_tab": pos_table_np.astype(np.float16),
            "idx_all": idx_all,
            "idx1": idx1,
            "recd": recd,
            "oh_all": oh_f.astype(np.float16),
        })
        meta.append(recs)
    return geom, ncol_total, in_maps, meta, tot_rows


def _build_nc(geom, ncol_total, tot_rows, kmax):
    from concourse import bass, bacc, mybir, library_config
    import concourse.tile as tile

    nrows = BPC * S
    nc = bacc.Bacc("TRN2", target_bir_lowering=False, debug=False)
    t_feats = nc.dram_tensor("feats_h", [nrows, D], mybir.dt.float16,
                             kind="ExternalInput")
    t_pos = nc.dram_tensor("pos_tab", [PV, D], mybir.dt.float16,
                           kind="ExternalInput")
    t_idx = nc.dram_tensor("idx_all", [P, 8 * ncol_total], mybir.dt.int16,
                           kind="ExternalInput")
    n1 = next(g[2] for g in geom if g[0] == 1)
    t_idx1 = nc.dram_tensor("idx1", [P, n1], mybir.dt.int32,
                            kind="ExternalInput")
    t_recd = nc.dram_tensor("recd", [P, ncol_total * P], mybir.dt.float16,
                            kind="ExternalInput")
    t_oh = nc.dram_tensor("oh_all", [PV, ncol_total * P], mybir.dt.float16,
                          kind="ExternalInput")
    t_out = nc.dram_tensor("out", [tot_rows, D], mybir.dt.float16,
                           kind="ExternalOutput")

    # overlapping row views: index i -> L*D consecutive elements (rows i..i+L-1)
    views = {}
    for (L, *_rest) in geom:
        views[L] = bass.AP(t_feats[:].tensor, 0,
                           [[D, nrows - L + 1], [1, L * D]])

    with tile.TileContext(nc) as tc:
        with (
            tc.tile_pool(name="const", bufs=1) as cpool,
            tc.tile_pool(name="gath", bufs=1) as gpool,
            tc.tile_pool(name="osb", bufs=4) as opool,
            tc.tile_pool(name="psum", bufs=4, space="PSUM") as ppool,
        ):
            # start the q7 dma_gather library fetch immediately; it loads
            # while the idx DMAs land and the library-free L=1 gathers run
            nc.gpsimd.load_library(library_config.mlp)
            idx1_sb = cpool.tile([P, n1], mybir.dt.int32)
            idx_sb = cpool.tile([P, 8 * ncol_total], mybir.dt.int16)
            recd_sb = cpool.tile([P, ncol_total * P], mybir.dt.float16)
            pos_sb = cpool.tile([PV, D], mybir.dt.float16)
            oh_sb = cpool.tile([PV, ncol_total * P], mybir.dt.float16)
            nc.sync.dma_start(out=idx1_sb[:], in_=t_idx1[:])
            nc.sync.dma_start(out=idx_sb[:], in_=t_idx[:])
            nc.sync.dma_start(out=pos_sb[:], in_=t_pos[:])
            nc.sync.dma_start(out=oh_sb[:], in_=t_oh[:])
            nc.sync.dma_start(out=recd_sb[:], in_=t_recd[:])

            gts = {}
            # L=1 first, on the library-free INDIRECT1D path: its descriptors
            # generate while the q7 dma_gather library is still being fetched
            for (L, cb, ncols, cu_list, rb, ms) in geom:
                if L != 1:
                    continue
                gt = gpool.tile([P, ncols, L * D], mybir.dt.float16,
                                tag=f"g{L}")
                gts[L] = gt
                for c in range(ncols):
                    nc.gpsimd.indirect_dma_start(
                        out=gt[:, c, :],
                        out_offset=None,
                        in_=t_feats[:],
                        in_offset=bass.IndirectOffsetOnAxis(
                            ap=idx1_sb[:, c:c + 1], axis=0),
                    )
            for (L, cb, ncols, cu_list, rb, ms) in geom:
                if L == 1:
                    continue
                gt = gpool.tile([P, ncols, L * D], mybir.dt.float16,
                                tag=f"g{L}")
                gts[L] = gt
                nc.gpsimd.dma_gather(
                    gt[:, :, :],
                    views[L],
                    idx_sb[:, 8 * cb: 8 * (cb + ncols)],
                    P * ncols,            # num_idxs (idx-buffer capacity)
                    ms,                   # num_idxs_reg (descriptors emitted)
                    L * D,
                    elem_step=D,
                    single_packet=False,
                )

            # flat column list in processing order
            colwork = []
            for (L, cb, ncols, cu_list, rb, ms) in geom:
                rowoff = rb
                for c in range(ncols):
                    colwork.append((L, cb + c, c, cu_list[c], rowoff))
                    rowoff += cu_list[c]

            pending = []          # (psum, cu, rowoff) awaiting drain, lag 2

            def drain_one():
                psum, cu, rowoff = pending.pop(0)
                osb = opool.tile([P, D], mybir.dt.float16, tag="osb")
                nc.scalar.activation(out=osb[0:cu, :], in_=psum[0:cu, :],
                                     func=mybir.ActivationFunctionType.Copy)
                nc.sync.dma_start(out=t_out[rowoff:rowoff + cu, :],
                                  in_=osb[0:cu, :])

            for (L, k, c, cu, rowoff) in colwork:
                gt = gts[L]

                def row(r):
                    return gt[0:cu, c, r * D:(r + 1) * D]

                step = 1           # pairwise in-place fold: result in row 0
                while step < L:
                    for i in range(0, L - step, 2 * step):
                        nc.vector.tensor_add(out=row(i), in0=row(i),
                                             in1=row(i + step))
                    step *= 2
                psum = ppool.tile([P, D], mybir.dt.float32, space="PSUM",
                                  tag="ps")
                lhs = oh_sb[:, k * P:k * P + cu]
                dg = recd_sb[0:cu, k * P:k * P + cu]
                nc.tensor.matmul(out=psum[0:cu, 0:512], lhsT=lhs,
                                 rhs=pos_sb[:, 0:512], start=True, stop=False)
                nc.tensor.matmul(out=psum[0:cu, 0:512], lhsT=dg,
                                 rhs=gt[0:cu, c, 0:512], start=False,
                                 stop=True)
                nc.tensor.matmul(out=psum[0:cu, 512:D], lhsT=lhs,
                                 rhs=pos_sb[:, 512:D], start=True, stop=False)
                nc.tensor.matmul(out=psum[0:cu, 512:D], lhsT=dg,
                                 rhs=gt[0:cu, c, 512:D], start=False,
                                 stop=True)
                pending.append((psum, cu, rowoff))
                if len(pending) > 1:
                    drain_one()
            while pending:
                drain_one()
    nc.finalize()
    return nc


def kernel(feats, pos_table, word_lens, pos, seq_len):
    global LAST_RESULTS
    feats = np.ascontiguousarray(np.asarray(feats, np.float32))
    pos_table_np = np.ascontiguousarray(np.asarray(pos_table, np.float32))
    starts, lens, recip, po = _word_ranges(word_lens, pos, seq_len)

    kmax = int(lens.max())
    shapes_ok = (
        feats.shape == (B, S, D)
        and pos_table_np.shape == (PV, D)
        and po.shape == (B, W)
        and starts.shape == (B, W)
        and np.asarray(seq_len).shape == (B,)
        and int(po.max()) < PV and int(po.min()) >= 0
    )
    if kmax > KMAX_DEVICE or kmax < 1 or not shapes_ok \
            or not _concourse_importable():
        return _numpy_fallback(feats, pos_table, word_lens, pos, seq_len)

    geom, ncol_total, in_maps, meta, tot_rows = _prepare(
        feats, pos_table_np, starts, lens, recip, po, kmax)
    nc = _build_nc(geom, ncol_total, tot_rows, kmax)

    res = _run_spmd(nc, in_maps, list(range(N_CORES)))
    LAST_RESULTS = res

    out = np.zeros((B, W, D), np.float32)
    for core in range(N_CORES):
        arr = res.results[core]["out"]            # [tot_rows, D]
        for bg, wsel, rowstart in meta[core]:
            out[bg][wsel] = arr[rowstart:rowstart + len(wsel)]
    # slots the device never computes: invalid words and len-0 words get
    # means == 0, so the exact answer is just the pos embedding row
    zmask = lens == 0
    if zmask.any():
        out[zmask] = pos_table_np[po[zmask]]
    return out
